# revision 10
# baseline (speedup 1.0000x reference)
# ContentLoss (cosine-similarity pairwise distance) Trainium2 kernel.
#
# Reference computation:
#   x1, x2: [B=4, C=256, W=256, H=256] f32; rand_int1/2: [n=256] indices into W*H
#   a1 = x1f[:, :, idx1], b1 = x1f[:, :, idx2]   (gather spatial columns)
#   D1 = cos_sim(a1, b1, axis=C), D2 likewise for x2
#   out = mean(|D1 - D2|)                        (scalar f32)
#
# Only the 2*n gathered spatial columns of each tensor are ever used, so the
# kernel avoids streaming the 512 MiB of input through the cores. Sharding
# (data-parallel over the 8 cores): core k handles (batch = k//2,
# tensor = x1 if k%2==0 else x2). The host hands each core its batch slice
# transposed to [W*H, C] so one gathered pixel is a contiguous 1 KiB row,
# and the replicated indices. On-device, per core:
#   - indirect DMA gather of the n idx1-rows and n idx2-rows (a, b tiles)
#   - dot = sum_C(a*b), saa = sum_C(a*a), sbb = sum_C(b*b) per gathered pixel
#     (single tensor_tensor_reduce instruction each on the vector engine)
# The host then finishes the O(B*n) scalar math: D = dot/max(sqrt(saa*sbb),
# eps) per (tensor, batch, pixel), and the final mean over |D1-D2|.

import numpy as np

B, C, W, H = 4, 256, 256, 256
S = W * H          # flattened spatial size
N = 256            # number of sampled pixel pairs (= W in the reference)
P = 128            # SBUF partitions
NCHUNK = N // P    # gather instructions per index set
EPS = 1e-8
N_CORES = 8

LAST_RESULTS = None  # BassKernelResults of the most recent run (for profiling)


def _build_nc():
    """Build the single-core Bass program (SPMD: same NEFF on all 8 cores).

    Inputs:  xt  [S, C] f32 — one (batch, tensor) slice, spatial-major
             idx [P, 2*NCHUNK] i32 — col j: idx1[j*128:(j+1)*128], then idx2
    Output:  out [P, 3*NCHUNK] f32 — cols [dot_j..., saa_j..., sbb_j...]
    """
    from contextlib import ExitStack

    import concourse.bass as bass
    from concourse import mybir

    f32 = mybir.dt.float32
    i32 = mybir.dt.int32
    nc = bass.Bass(target_bir_lowering=False, debug=False)
    xt = nc.dram_tensor("xt", [S, C], f32, kind="ExternalInput")
    idx = nc.dram_tensor("idx", [P, 2 * NCHUNK], i32, kind="ExternalInput")
    out = nc.dram_tensor("out", [3 * NCHUNK, P], f32, kind="ExternalOutput")

    # Raw Bass (no Tile): this walrus build allows only one sync wait per
    # instruction, which Tile's drain/barrier tail violates; the manual
    # schedule below needs at most one wait anywhere and has no tail cost.
    # res columns are stored in order ORDER[i] -> out[:, q*NCHUNK+j].
    order = [(q, j) for j in range(NCHUNK) for q in (1, 2, 0)]

    with ExitStack() as stack:
        ec = stack.enter_context
        idx_sb = ec(nc.sbuf_tensor("idx_sb", [P, 2 * NCHUNK], i32))
        ga = [ec(nc.sbuf_tensor(f"ga{j}", [P, C], f32)) for j in range(NCHUNK)]
        gb = [ec(nc.sbuf_tensor(f"gb{j}", [P, C], f32)) for j in range(NCHUNK)]
        prods = {
            (q, j): ec(nc.sbuf_tensor(f"prod{q}_{j}", [P, C], f32)) for q, j in order
        }
        accs = {
            (q, j): ec(nc.sbuf_tensor(f"acc{q}_{j}", [P, 1], f32)) for q, j in order
        }
        s_idx = ec(nc.semaphore("s_idx"))
        s_v = ec(nc.semaphore("s_v"))
        s_acc = ec(nc.semaphore("s_acc"))

        # one completion sem per gather: multiple DMAs on a shared sem make
        # intermediate thresholds meaningless (16 SDMA engines inc by 1 each,
        # interleaved across DMAs)
        gathers = []
        for j in range(NCHUNK):
            gathers.append((ga[j], j))
            gathers.append((gb[j], NCHUNK + j))
        s_gs = [ec(nc.semaphore(f"s_g{i}")) for i in range(len(gathers))]
        g_sem = {t.name: s for (t, _), s in zip(gathers, s_gs)}
        block = ec(nc.Block())

        @block.gpsimd
        def _(gpsimd):
            gpsimd.wait_ge(s_idx, 16)
            for (tile_, col), s in zip(gathers, s_gs):
                gpsimd.indirect_dma_start(
                    out=tile_[:],
                    out_offset=None,
                    in_=xt[:],
                    in_offset=bass.IndirectOffsetOnAxis(
                        ap=idx_sb[:, col : col + 1], axis=0
                    ),
                ).then_inc(s, 16)

        @block.vector
        def _(vector):
            # DVE has no same-engine interlock: the reduce must wait on its
            # producing multiply via a sem. Every compute op incs s_v by 1.
            waited = set()
            vcnt = 0
            for j in range(NCHUNK):
                a, b = ga[j], gb[j]
                for q, (u, v) in ((1, (a, a)), (2, (b, b)), (0, (a, b))):
                    for t in (u, v):
                        if t.name not in waited:
                            vector.wait_ge(g_sem[t.name], 16)
                            waited.add(t.name)
                    vector.tensor_tensor(
                        out=prods[(q, j)][:], in0=u[:], in1=v[:], op=mybir.AluOpType.mult
                    ).then_inc(s_v, 1)
                    vcnt += 1
                    vector.wait_ge(s_v, vcnt)
                    vector.tensor_reduce(
                        out=accs[(q, j)][:],
                        in_=prods[(q, j)][:],
                        axis=mybir.AxisListType.X,
                        op=mybir.AluOpType.add,
                    ).then_inc(s_v, 1)
                    vcnt += 1

        @block.sync
        def _(sync):
            sync.dma_start(out=idx_sb[:], in_=idx[:]).then_inc(s_idx, 16)
            for i, (q, j) in enumerate(order):
                # acc (q, j) is ready once its reduce (vector op 2*(i+1)) ran
                sync.wait_ge(s_v, 2 * (i + 1))
                sync.dma_start(
                    out=out[q * NCHUNK + j, :],
                    in_=accs[(q, j)][:],
                ).then_inc(s_acc, 16)
            sync.wait_ge(s_acc, 16 * len(order))

    return nc


def _transpose_cs(x):
    """[C, S] f32 contiguous -> [S, C] contiguous, cache-blocked."""
    out = np.empty((S, C), np.float32)
    bs = 4096
    for s0 in range(0, S, bs):
        out[s0 : s0 + bs] = x[:, s0 : s0 + bs].T
    return out


def _ensure_ntff_hook():
    """Make `antenv.axon_hooks` importable (bass_utils needs it when tracing).

    Some images lack the module; provide a shim and, when possible, register
    the real ctypes NTFF hook so BASS_TRACE=1 profiling works.
    """
    try:
        import antenv.axon_hooks  # noqa: F401

        return
    except ImportError:
        pass
    import sys
    import types

    try:
        import antenv
    except ImportError:
        return
    m = types.ModuleType("antenv.axon_hooks")
    m._hook = None
    m.set_axon_ntff_profile_hook = lambda h: setattr(m, "_hook", h)
    m.get_axon_ntff_profile_hook = lambda: m._hook
    sys.modules["antenv.axon_hooks"] = m
    antenv.axon_hooks = m
    try:
        from trn_agent_boot.trn_boot import _ntff_profile_via_ctypes

        m._hook = _ntff_profile_via_ctypes("/opt/axon/libaxon_pjrt.so")
    except Exception:
        pass


def kernel(x1, x2, rand_int1, rand_int2):
    global LAST_RESULTS
    from concurrent.futures import ThreadPoolExecutor

    _ensure_ntff_hook()
    from concourse.bass_utils import run_bass_kernel_spmd

    x1 = np.ascontiguousarray(np.asarray(x1, dtype=np.float32)).reshape(B, C, S)
    x2 = np.ascontiguousarray(np.asarray(x2, dtype=np.float32)).reshape(B, C, S)
    idx1 = np.asarray(rand_int1).astype(np.int64)
    idx2 = np.asarray(rand_int2).astype(np.int64)
    assert idx1.shape == (N,) and idx2.shape == (N,)
    assert (0 <= idx1).all() and (idx1 < S).all()
    assert (0 <= idx2).all() and (idx2 < S).all()

    idxcols = np.empty((P, 2 * NCHUNK), np.int32)
    for j in range(NCHUNK):
        idxcols[:, j] = idx1[j * P : (j + 1) * P]
        idxcols[:, NCHUNK + j] = idx2[j * P : (j + 1) * P]

    # Shard: core k <- (batch k//2, tensor k%2), spatial-major layout.
    def make_in(k):
        b, t = divmod(k, 2)
        return {"xt": _transpose_cs((x1 if t == 0 else x2)[b]), "idx": idxcols}

    with ThreadPoolExecutor(max_workers=N_CORES) as ex:
        in_maps = list(ex.map(make_in, range(N_CORES)))

    nc = _build_nc()
    LAST_RESULTS = run_bass_kernel_spmd(nc, in_maps, core_ids=list(range(N_CORES)))

    # Unshard: finish the cosine + mean in f64 on host.
    D = np.empty((2, B, N), np.float64)
    for k, r in enumerate(LAST_RESULTS.results):
        b, t = divmod(k, 2)
        o = r["out"].astype(np.float64)
        dot = o[0:NCHUNK].reshape(N)  # row j, col p -> i = j*128 + p
        saa = o[NCHUNK : 2 * NCHUNK].reshape(N)
        sbb = o[2 * NCHUNK : 3 * NCHUNK].reshape(N)
        D[t, b] = dot / np.maximum(np.sqrt(saa * sbb), EPS)
    return np.array(np.mean(np.abs(D[0] - D[1])), dtype=np.float32)


# revision 14
# speedup vs baseline: 1.5202x; 1.5202x over previous
# ContentLoss (cosine-similarity pairwise distance) Trainium2 kernel.
#
# Reference computation:
#   x1, x2: [B=4, C=256, W=256, H=256] f32; rand_int1/2: [n=256] indices into W*H
#   a1 = x1f[:, :, idx1], b1 = x1f[:, :, idx2]   (gather spatial columns)
#   D1 = cos_sim(a1, b1, axis=C), D2 likewise for x2
#   out = mean(|D1 - D2|)                        (scalar f32)
#
# Only the 2*n gathered spatial columns of each tensor are ever used, so the
# kernel avoids streaming the 512 MiB of input through the cores. Sharding
# (data-parallel over the 8 cores): core k handles (batch = k//2,
# tensor = x1 if k%2==0 else x2). The host hands each core its batch slice
# transposed to [W*H, C] so one gathered pixel is a contiguous 1 KiB row,
# and the replicated indices. On-device, per core:
#   - indirect DMA gather of the n idx1-rows and n idx2-rows (a, b tiles)
#   - dot = sum_C(a*b), saa = sum_C(a*a), sbb = sum_C(b*b) per gathered pixel
#     (single tensor_tensor_reduce instruction each on the vector engine)
# The host then finishes the O(B*n) scalar math: D = dot/max(sqrt(saa*sbb),
# eps) per (tensor, batch, pixel), and the final mean over |D1-D2|.

import numpy as np

B, C, W, H = 4, 256, 256, 256
S = W * H          # flattened spatial size
N = 256            # number of sampled pixel pairs (= W in the reference)
P = 128            # SBUF partitions
NCHUNK = N // P    # gather instructions per index set
EPS = 1e-8
N_CORES = 8

LAST_RESULTS = None  # BassKernelResults of the most recent run (for profiling)


def _build_nc():
    """Build the single-core Bass program (SPMD: same NEFF on all 8 cores).

    Inputs:  xt  [S, C] f32 — one (batch, tensor) slice, spatial-major
             idx [P, 2*NCHUNK] i32 — col j: idx1[j*128:(j+1)*128], then idx2
    Output:  out [P, 3*NCHUNK] f32 — cols [dot_j..., saa_j..., sbb_j...]
    """
    from contextlib import ExitStack

    import concourse.bass as bass
    from concourse import mybir

    f32 = mybir.dt.float32
    i32 = mybir.dt.int32
    # 4 SWDGE queues: one per indirect gather, so the four descriptor rings
    # drain in parallel (each SDMA engine interleaves rings at packet
    # granularity -> 4x outstanding HBM reads). Scratch sized to hold all
    # descriptor pairs at once so Q7 never stalls waiting for ring space.
    nc = bass.Bass(
        target_bir_lowering=False,
        debug=False,
        num_swdge_queues=4,
        dynamic_dma_scratch_size=65536,
    )
    xt = nc.dram_tensor("xt", [S, C], f32, kind="ExternalInput")
    idx = nc.dram_tensor("idx", [P, 2 * NCHUNK], i32, kind="ExternalInput")
    out = nc.dram_tensor("out", [P, 3 * NCHUNK], f32, kind="ExternalOutput")

    # Raw Bass (no Tile): this walrus build allows only one sync wait per
    # instruction, which Tile's drain/barrier tail violates; the manual
    # schedule below needs at most one wait anywhere and has no tail cost.
    # idx columns are [a0, b0, a1, b1] = [idx1_j0, idx2_j0, idx1_j1, idx2_j1].
    order = [(q, j) for j in range(NCHUNK) for q in (1, 2, 0)]

    with ExitStack() as stack:
        ec = stack.enter_context
        idx_sb = ec(nc.sbuf_tensor("idx_sb", [P, 2 * NCHUNK], i32))
        ga = [ec(nc.sbuf_tensor(f"ga{j}", [P, C], f32)) for j in range(NCHUNK)]
        gb = [ec(nc.sbuf_tensor(f"gb{j}", [P, C], f32)) for j in range(NCHUNK)]
        prods = {
            (q, j): ec(nc.sbuf_tensor(f"prod{q}_{j}", [P, C], f32)) for q, j in order
        }
        acc = ec(nc.sbuf_tensor("acc", [P, 3 * NCHUNK], f32))
        s_idx = ec(nc.semaphore("s_idx"))
        s_v = ec(nc.semaphore("s_v"))
        s_acc = ec(nc.semaphore("s_acc"))
        # one completion sem per gather: multiple DMAs on a shared sem make
        # intermediate thresholds meaningless (16 SDMA engines inc by 1 each,
        # interleaved across DMAs)
        gathers = []  # (dst tile, idx column) in issue order: a0, b0, a1, b1
        for j in range(NCHUNK):
            gathers.append((ga[j], 2 * j))
            gathers.append((gb[j], 2 * j + 1))
        s_gs = [ec(nc.semaphore(f"s_g{i}")) for i in range(len(gathers))]
        g_sem = {t.name: s for (t, _), s in zip(gathers, s_gs)}
        block = ec(nc.Block())

        @block.gpsimd
        def _(gpsimd):
            gpsimd.dma_start(out=idx_sb[:], in_=idx[:]).then_inc(s_idx, 16)
            gpsimd.wait_ge(s_idx, 16)
            for i, ((tile_, col), s) in enumerate(zip(gathers, s_gs)):
                inst = gpsimd.indirect_dma_start(
                    out=tile_[:],
                    out_offset=None,
                    in_=xt[:],
                    in_offset=bass.IndirectOffsetOnAxis(
                        ap=idx_sb[:, col : col + 1], axis=0
                    ),
                )
                qn = i % nc.num_swdge_queues
                inst.ins.queue = f"qPoolDynamic{qn or ''}"
                inst.then_inc(s, 16)

        @block.vector
        def _(vector):
            # DVE has no same-engine interlock: the reduce must wait on its
            # producing multiply via a sem. Every compute op incs s_v by 1.
            vcnt = 0
            waited = set()
            for j in range(NCHUNK):
                a, b = ga[j], gb[j]
                for q, (u, v) in ((1, (a, a)), (2, (b, b)), (0, (a, b))):
                    for t in (u, v):
                        if t.name not in waited:
                            vector.wait_ge(g_sem[t.name], 16)
                            waited.add(t.name)
                    vector.tensor_tensor(
                        out=prods[(q, j)][:], in0=u[:], in1=v[:], op=mybir.AluOpType.mult
                    ).then_inc(s_v, 1)
                    vcnt += 1
                    vector.wait_ge(s_v, vcnt)
                    vector.tensor_reduce(
                        out=acc[:, q * NCHUNK + j : q * NCHUNK + j + 1],
                        in_=prods[(q, j)][:],
                        axis=mybir.AxisListType.X,
                        op=mybir.AluOpType.add,
                    ).then_inc(s_v, 1)
                    vcnt += 1

        @block.sync
        def _(sync):
            sync.wait_ge(s_v, 2 * len(order))
            sync.dma_start(out=out[:], in_=acc[:]).then_inc(s_acc, 16)
            sync.wait_ge(s_acc, 16)

    return nc


def _transpose_cs(x):
    """[C, S] f32 contiguous -> [S, C] contiguous, cache-blocked."""
    out = np.empty((S, C), np.float32)
    bs = 4096
    for s0 in range(0, S, bs):
        out[s0 : s0 + bs] = x[:, s0 : s0 + bs].T
    return out


def _ensure_ntff_hook():
    """Make `antenv.axon_hooks` importable (bass_utils needs it when tracing).

    Some images lack the module; provide a shim and, when possible, register
    the real ctypes NTFF hook so BASS_TRACE=1 profiling works.
    """
    try:
        import antenv.axon_hooks  # noqa: F401

        return
    except ImportError:
        pass
    import sys
    import types

    try:
        import antenv
    except ImportError:
        return
    m = types.ModuleType("antenv.axon_hooks")
    m._hook = None
    m.set_axon_ntff_profile_hook = lambda h: setattr(m, "_hook", h)
    m.get_axon_ntff_profile_hook = lambda: m._hook
    sys.modules["antenv.axon_hooks"] = m
    antenv.axon_hooks = m
    try:
        from trn_agent_boot.trn_boot import _ntff_profile_via_ctypes

        m._hook = _ntff_profile_via_ctypes("/opt/axon/libaxon_pjrt.so")
    except Exception:
        pass


def kernel(x1, x2, rand_int1, rand_int2):
    global LAST_RESULTS
    from concurrent.futures import ThreadPoolExecutor

    _ensure_ntff_hook()
    from concourse.bass_utils import run_bass_kernel_spmd

    x1 = np.ascontiguousarray(np.asarray(x1, dtype=np.float32)).reshape(B, C, S)
    x2 = np.ascontiguousarray(np.asarray(x2, dtype=np.float32)).reshape(B, C, S)
    idx1 = np.asarray(rand_int1).astype(np.int64)
    idx2 = np.asarray(rand_int2).astype(np.int64)
    assert idx1.shape == (N,) and idx2.shape == (N,)
    assert (0 <= idx1).all() and (idx1 < S).all()
    assert (0 <= idx2).all() and (idx2 < S).all()

    # The mean over pairs is order-invariant, so sort pairs by idx1: the
    # a-gathers then walk HBM in address order (row-buffer locality).
    perm = np.argsort(idx1, kind="stable")
    idx1 = idx1[perm]
    idx2 = idx2[perm]

    idxcols = np.empty((P, 2 * NCHUNK), np.int32)
    for j in range(NCHUNK):
        idxcols[:, 2 * j] = idx1[j * P : (j + 1) * P]
        idxcols[:, 2 * j + 1] = idx2[j * P : (j + 1) * P]

    # Shard: core k <- (batch k//2, tensor k%2), spatial-major layout.
    def make_in(k):
        b, t = divmod(k, 2)
        return {"xt": _transpose_cs((x1 if t == 0 else x2)[b]), "idx": idxcols}

    with ThreadPoolExecutor(max_workers=N_CORES) as ex:
        in_maps = list(ex.map(make_in, range(N_CORES)))

    nc = _build_nc()
    LAST_RESULTS = run_bass_kernel_spmd(nc, in_maps, core_ids=list(range(N_CORES)))

    # Unshard: finish the cosine + mean in f64 on host.
    D = np.empty((2, B, N), np.float64)
    for k, r in enumerate(LAST_RESULTS.results):
        b, t = divmod(k, 2)
        o = r["out"].astype(np.float64)
        dot = o[:, 0:NCHUNK].T.reshape(N)  # col j, row p -> i = j*128 + p
        saa = o[:, NCHUNK : 2 * NCHUNK].T.reshape(N)
        sbb = o[:, 2 * NCHUNK : 3 * NCHUNK].T.reshape(N)
        D[t, b] = dot / np.maximum(np.sqrt(saa * sbb), EPS)
    return np.array(np.mean(np.abs(D[0] - D[1])), dtype=np.float32)


# revision 15
# speedup vs baseline: 1.5440x; 1.0157x over previous
# ContentLoss (cosine-similarity pairwise distance) Trainium2 kernel.
#
# Reference computation:
#   x1, x2: [B=4, C=256, W=256, H=256] f32; rand_int1/2: [n=256] indices into W*H
#   a1 = x1f[:, :, idx1], b1 = x1f[:, :, idx2]   (gather spatial columns)
#   D1 = cos_sim(a1, b1, axis=C), D2 likewise for x2
#   out = mean(|D1 - D2|)                        (scalar f32)
#
# Only the 2*n gathered spatial columns of each tensor are ever used, so the
# kernel avoids streaming the 512 MiB of input through the cores. Sharding
# (data-parallel over the 8 cores): core k handles (batch = k//2,
# tensor = x1 if k%2==0 else x2). The host hands each core its batch slice
# transposed to [W*H, C] so one gathered pixel is a contiguous 1 KiB row,
# and the replicated indices. On-device, per core:
#   - indirect DMA gather of the n idx1-rows and n idx2-rows (a, b tiles)
#   - dot = sum_C(a*b), saa = sum_C(a*a), sbb = sum_C(b*b) per gathered pixel
#     (single tensor_tensor_reduce instruction each on the vector engine)
# The host then finishes the O(B*n) scalar math: D = dot/max(sqrt(saa*sbb),
# eps) per (tensor, batch, pixel), and the final mean over |D1-D2|.

import numpy as np

B, C, W, H = 4, 256, 256, 256
S = W * H          # flattened spatial size
N = 256            # number of sampled pixel pairs (= W in the reference)
P = 128            # SBUF partitions
NCHUNK = N // P    # gather instructions per index set
EPS = 1e-8
N_CORES = 8

LAST_RESULTS = None  # BassKernelResults of the most recent run (for profiling)


def _build_nc():
    """Build the single-core Bass program (SPMD: same NEFF on all 8 cores).

    Inputs:  xt  [S, C] f32 — one (batch, tensor) slice, spatial-major
             idx [P, 2*NCHUNK] i32 — col j: idx1[j*128:(j+1)*128], then idx2
    Output:  out [P, 3*NCHUNK] f32 — cols [dot_j..., saa_j..., sbb_j...]
    """
    from contextlib import ExitStack

    import concourse.bass as bass
    from concourse import mybir

    f32 = mybir.dt.float32
    i32 = mybir.dt.int32
    # 4 SWDGE queues: one per indirect gather, so the four descriptor rings
    # drain in parallel (each SDMA engine interleaves rings at packet
    # granularity -> 4x outstanding HBM reads). Scratch sized to hold all
    # descriptor pairs at once so Q7 never stalls waiting for ring space.
    nc = bass.Bass(
        target_bir_lowering=False,
        debug=False,
        num_swdge_queues=4,
        dynamic_dma_scratch_size=65536,
    )
    xt = nc.dram_tensor("xt", [S, C], f32, kind="ExternalInput")
    idx = nc.dram_tensor("idx", [P, 2 * NCHUNK], i32, kind="ExternalInput")
    out = nc.dram_tensor("out", [P, 3 * NCHUNK], f32, kind="ExternalOutput")

    # Raw Bass (no Tile): this walrus build allows only one sync wait per
    # instruction, which Tile's drain/barrier tail violates; the manual
    # schedule below needs at most one wait anywhere and has no tail cost.
    # idx columns are [a0, b0, a1, b1] = [idx1_j0, idx2_j0, idx1_j1, idx2_j1].
    order = [(q, j) for j in range(NCHUNK) for q in (1, 2, 0)]

    with ExitStack() as stack:
        ec = stack.enter_context
        idx_sb = ec(nc.sbuf_tensor("idx_sb", [P, 2 * NCHUNK], i32))
        ga = [ec(nc.sbuf_tensor(f"ga{j}", [P, C], f32)) for j in range(NCHUNK)]
        gb = [ec(nc.sbuf_tensor(f"gb{j}", [P, C], f32)) for j in range(NCHUNK)]
        prods = {
            (q, j): ec(nc.sbuf_tensor(f"prod{q}_{j}", [P, C], f32)) for q, j in order
        }
        acc = ec(nc.sbuf_tensor("acc", [P, 3 * NCHUNK], f32))
        s_idx = ec(nc.semaphore("s_idx"))
        s_v = ec(nc.semaphore("s_v"))
        s_acc = ec(nc.semaphore("s_acc"))
        # one completion sem per gather: multiple DMAs on a shared sem make
        # intermediate thresholds meaningless (16 SDMA engines inc by 1 each,
        # interleaved across DMAs)
        gathers = []  # (dst tile, idx column) in issue order: a0, b0, a1, b1
        for j in range(NCHUNK):
            gathers.append((ga[j], 2 * j))
            gathers.append((gb[j], 2 * j + 1))
        s_gs = [ec(nc.semaphore(f"s_g{i}")) for i in range(len(gathers))]
        g_sem = {t.name: s for (t, _), s in zip(gathers, s_gs)}
        block = ec(nc.Block(no_gpsimd_drain=True))

        @block.gpsimd
        def _(gpsimd):
            gpsimd.wait_ge(s_idx, 16)
            for i, ((tile_, col), s) in enumerate(zip(gathers, s_gs)):
                inst = gpsimd.indirect_dma_start(
                    out=tile_[:],
                    out_offset=None,
                    in_=xt[:],
                    in_offset=bass.IndirectOffsetOnAxis(
                        ap=idx_sb[:, col : col + 1], axis=0
                    ),
                )
                qn = i % nc.num_swdge_queues
                inst.ins.queue = f"qPoolDynamic{qn or ''}"
                inst.then_inc(s, 16)

        @block.vector
        def _(vector):
            # DVE has no same-engine interlock: the reduce must wait on its
            # producing multiply via a sem. Every compute op incs s_v by 1.
            vcnt = 0
            waited = set()
            for j in range(NCHUNK):
                a, b = ga[j], gb[j]
                for q, (u, v) in ((1, (a, a)), (2, (b, b)), (0, (a, b))):
                    for t in (u, v):
                        if t.name not in waited:
                            vector.wait_ge(g_sem[t.name], 16)
                            waited.add(t.name)
                    vector.tensor_tensor(
                        out=prods[(q, j)][:], in0=u[:], in1=v[:], op=mybir.AluOpType.mult
                    ).then_inc(s_v, 1)
                    vcnt += 1
                    vector.wait_ge(s_v, vcnt)
                    vector.tensor_reduce(
                        out=acc[:, q * NCHUNK + j : q * NCHUNK + j + 1],
                        in_=prods[(q, j)][:],
                        axis=mybir.AxisListType.X,
                        op=mybir.AluOpType.add,
                    ).then_inc(s_v, 1)
                    vcnt += 1

        @block.sync
        def _(sync):
            # sync's preamble retires ~2us before gpsimd's, so it issues the
            # idx load; gpsimd just waits on the completion sem
            sync.dma_start(out=idx_sb[:], in_=idx[:]).then_inc(s_idx, 16)
            sync.wait_ge(s_v, 2 * len(order))
            sync.dma_start(out=out[:], in_=acc[:]).then_inc(s_acc, 16)
            sync.wait_ge(s_acc, 16)

    return nc


def _transpose_cs(x):
    """[C, S] f32 contiguous -> [S, C] contiguous, cache-blocked."""
    out = np.empty((S, C), np.float32)
    bs = 4096
    for s0 in range(0, S, bs):
        out[s0 : s0 + bs] = x[:, s0 : s0 + bs].T
    return out


def _ensure_ntff_hook():
    """Make `antenv.axon_hooks` importable (bass_utils needs it when tracing).

    Some images lack the module; provide a shim and, when possible, register
    the real ctypes NTFF hook so BASS_TRACE=1 profiling works.
    """
    try:
        import antenv.axon_hooks  # noqa: F401

        return
    except ImportError:
        pass
    import sys
    import types

    try:
        import antenv
    except ImportError:
        return
    m = types.ModuleType("antenv.axon_hooks")
    m._hook = None
    m.set_axon_ntff_profile_hook = lambda h: setattr(m, "_hook", h)
    m.get_axon_ntff_profile_hook = lambda: m._hook
    sys.modules["antenv.axon_hooks"] = m
    antenv.axon_hooks = m
    try:
        from trn_agent_boot.trn_boot import _ntff_profile_via_ctypes

        m._hook = _ntff_profile_via_ctypes("/opt/axon/libaxon_pjrt.so")
    except Exception:
        pass


def kernel(x1, x2, rand_int1, rand_int2):
    global LAST_RESULTS
    from concurrent.futures import ThreadPoolExecutor

    _ensure_ntff_hook()
    from concourse.bass_utils import run_bass_kernel_spmd

    x1 = np.ascontiguousarray(np.asarray(x1, dtype=np.float32)).reshape(B, C, S)
    x2 = np.ascontiguousarray(np.asarray(x2, dtype=np.float32)).reshape(B, C, S)
    idx1 = np.asarray(rand_int1).astype(np.int64)
    idx2 = np.asarray(rand_int2).astype(np.int64)
    assert idx1.shape == (N,) and idx2.shape == (N,)
    assert (0 <= idx1).all() and (idx1 < S).all()
    assert (0 <= idx2).all() and (idx2 < S).all()

    # The mean over pairs is order-invariant, so sort pairs by idx1: the
    # a-gathers then walk HBM in address order (row-buffer locality).
    perm = np.argsort(idx1, kind="stable")
    idx1 = idx1[perm]
    idx2 = idx2[perm]

    idxcols = np.empty((P, 2 * NCHUNK), np.int32)
    for j in range(NCHUNK):
        idxcols[:, 2 * j] = idx1[j * P : (j + 1) * P]
        idxcols[:, 2 * j + 1] = idx2[j * P : (j + 1) * P]

    # Shard: core k <- (batch k//2, tensor k%2), spatial-major layout.
    def make_in(k):
        b, t = divmod(k, 2)
        return {"xt": _transpose_cs((x1 if t == 0 else x2)[b]), "idx": idxcols}

    with ThreadPoolExecutor(max_workers=N_CORES) as ex:
        in_maps = list(ex.map(make_in, range(N_CORES)))

    nc = _build_nc()
    LAST_RESULTS = run_bass_kernel_spmd(nc, in_maps, core_ids=list(range(N_CORES)))

    # Unshard: finish the cosine + mean in f64 on host.
    D = np.empty((2, B, N), np.float64)
    for k, r in enumerate(LAST_RESULTS.results):
        b, t = divmod(k, 2)
        o = r["out"].astype(np.float64)
        dot = o[:, 0:NCHUNK].T.reshape(N)  # col j, row p -> i = j*128 + p
        saa = o[:, NCHUNK : 2 * NCHUNK].T.reshape(N)
        sbb = o[:, 2 * NCHUNK : 3 * NCHUNK].T.reshape(N)
        D[t, b] = dot / np.maximum(np.sqrt(saa * sbb), EPS)
    return np.array(np.mean(np.abs(D[0] - D[1])), dtype=np.float32)
